# revision 12
# baseline (speedup 1.0000x reference)
# Trainium2 Bass kernel for nn_Attention_48352741818760 (gnn_message_passing).
#
# Sharding: tensor-parallel over the 8 attention heads — one head per
# NeuronCore.  Each core computes its head's q/k/v projections, the folded
# edge linears (edge gather on device via dma_gather), the joint [4096 x 4096]
# softmax attention for its head, and its head's slice of the output
# projection.  The host sums the 8 partial [4096, 128] outputs and adds the
# final biases.
#
# Host-side preprocessing is restricted to layout changes (bf16 casts,
# transposes, zero padding, int16 index wrapping) and constant folding of the
# model weights (scales ALPHA, c, 1/sqrt(2), DH^-0.5 folded into the weight
# matrices; the k/q/v endpoint-sum terms folded into the edge linears via
# k[src]+k[dst] == (nodes[src]+nodes[dst]) @ Wk + 2 bk).
#
# Device numerics: all matmuls in bf16 with fp32 PSUM accumulation; softmax
# in fp32 without max subtraction (scores are bounded, |S| < ~6, checked on
# host against the reference); row sums via a ones-column appended to V;
# normalization applied after the output projection via per-partition
# reciprocal scaling.
#
# All PSUM accumulation groups use a uniform contraction size of 128 (the
# EFD=64 edge-feature contraction is zero-padded to 128) — mixed-K groups
# break PSUM has_written semantics.

import sys

if "/opt/trn_rl_repo" not in sys.path:
    sys.path.insert(0, "/opt/trn_rl_repo")

import numpy as np
import ml_dtypes

DIM = 128
DH = 64
H = 8
INNER = 512
EFD = 64
N = 1024
E = 3072
T = N + E  # 4096
ALPHA = 0.8
N_CORES = 8

_BF16 = ml_dtypes.bfloat16

_PROGRAM = None


def _build_program():
    import concourse.bacc as bacc
    import concourse.bass as bass
    import concourse.tile as tile
    from concourse import mybir
    from concourse.masks import make_identity

    F32 = mybir.dt.float32
    BF16 = mybir.dt.bfloat16
    I32 = mybir.dt.int32
    AF = mybir.ActivationFunctionType
    ADD = mybir.AluOpType.add

    nc = bacc.Bacc(
        "TRN2",
        target_bir_lowering=False,
        debug=False,
        enable_asserts=False,
        num_devices=N_CORES,
    )

    d_nodes = nc.dram_tensor("nodes_bf", [N, DIM], BF16, kind="ExternalInput").ap()
    d_efT = nc.dram_tensor("efT_bf", [DIM, E], BF16, kind="ExternalInput").ap()
    d_idx = nc.dram_tensor("idx_i32", [2, 128, E // 128], I32, kind="ExternalInput").ap()
    d_wqkv = nc.dram_tensor("wqkv", [DIM, 3 * DH], BF16, kind="ExternalInput").ap()
    d_eAB = nc.dram_tensor("eAB", [DIM, 6 * DH], BF16, kind="ExternalInput").ap()
    d_eC = nc.dram_tensor("eC", [DIM, 3 * DH], BF16, kind="ExternalInput").ap()
    d_wout = nc.dram_tensor("wout", [DH, 2 * DIM], BF16, kind="ExternalInput").ap()
    d_bias = nc.dram_tensor("bias6", [DH, 6], F32, kind="ExternalInput").ap()
    d_out = nc.dram_tensor("out_partial", [T, DIM], F32, kind="ExternalOutput").ap()

    NJ = T // 128   # 32 j-chunks of 128 context tokens
    NM = T // 512   # 8 m-chunks of 512 query tokens
    DH1 = DH + 1

    from contextlib import ExitStack

    with tile.TileContext(nc) as tc, ExitStack() as ctx:
        const = ctx.enter_context(tc.tile_pool(name="const", bufs=1))
        big = ctx.enter_context(tc.tile_pool(name="big", bufs=1))
        work = ctx.enter_context(tc.tile_pool(name="work", bufs=3))
        ptp = ctx.enter_context(tc.tile_pool(name="pt", bufs=18))
        stp = ctx.enter_context(tc.tile_pool(name="st", bufs=2, space="PSUM"))
        avp = ctx.enter_context(tc.tile_pool(name="av", bufs=2, space="PSUM"))
        smp = ctx.enter_context(tc.tile_pool(name="sm", bufs=2, space="PSUM"))

        # ---- constants / weights ----
        wqkv = const.tile([DIM, 3 * DH], BF16, name="wqkv")
        nc.sync.dma_start(wqkv[:], d_wqkv)
        eAB = const.tile([DIM, 6 * DH], BF16, name="eAB")
        nc.sync.dma_start(eAB[:], d_eAB)
        eC = const.tile([DIM, 3 * DH], BF16, name="eC")
        nc.sync.dma_start(eC[:], d_eC)
        wout = const.tile([DH, 2 * DIM], BF16, name="wout")
        nc.sync.dma_start(wout[:], d_wout)
        bias6 = const.tile([DH, 6], F32, name="bias6")
        nc.sync.dma_start(bias6[:], d_bias)
        ones = const.tile([1, 8], F32, name="ones")
        nc.vector.memset(ones[:], 1.0)
        ident = const.tile([128, 128], BF16, name="ident")
        make_identity(nc, ident[:])

        idx_src = const.tile([128, E // 128], I32, name="idx_src")
        nc.sync.dma_start(idx_src[:], d_idx[0])
        idx_dst = const.tile([128, E // 128], I32, name="idx_dst")
        nc.sync.dma_start(idx_dst[:], d_idx[1])

        import os
        _no_gather = "nogather" in os.environ.get("BASS_VARIANT", "")
        _no_dmat = "nodmat" in os.environ.get("BASS_VARIANT", "")
        _no_sb2sb = "nosb2sb" in os.environ.get("BASS_VARIANT", "")

        # ---- gathered endpoint features (transposed) ----
        # indirect row gather (128 edge rows per call) + PE transpose to get
        # the features onto partitions.
        srcT = big.tile([128, E], BF16, name="srcT")
        dstT = big.tile([128, E], BF16, name="dstT")
        if _no_gather:
            nc.vector.memset(srcT[:], 0.0)
            nc.vector.memset(dstT[:], 0.0)
        else:
            for x, (idxt, dest) in enumerate(((idx_src, srcT), (idx_dst, dstT))):
                for ch in range(E // 128):
                    gsb = work.tile([128, DIM], BF16, tag="gsb", name=f"g_{x}_{ch}")
                    nc.gpsimd.indirect_dma_start(
                        out=gsb[:], out_offset=None, in_=d_nodes,
                        in_offset=bass.IndirectOffsetOnAxis(
                            ap=idxt[:, ch:ch + 1], axis=0),
                    )
                    gt = smp.tile([128, DIM], BF16, tag="sm", name=f"gt_{x}_{ch}")
                    nc.tensor.transpose(gt[:], gsb[:], ident[:])
                    nc.vector.tensor_copy(dest[:, ch * 128:(ch + 1) * 128], gt[:])

        nodesT = big.tile([DIM, N], BF16, name="nodesT")
        if _no_dmat:
            nc.vector.memset(nodesT[:], 0.0)
        else:
            nc.sync.dma_start_transpose(nodesT[:], d_nodes)
        efT = big.tile([DIM, E], BF16, name="efT")
        nc.sync.dma_start(efT[:], d_efT)

        # ---- persistent tensors ----
        QT = big.tile([128, T], BF16, name="QT")
        KT = big.tile([128, T], BF16, name="KT")
        vT = big.tile([DH, T], BF16, name="vT")
        Vn = big.tile([128, NJ * DH1], BF16, name="Vn")
        OT = big.tile([DH, T], BF16, name="OT")
        rrow = big.tile([1, T], F32, name="rrow")
        rcp = big.tile([1, T], F32, name="rcp")

        def proj_copy(ps, dest, dsl, bcol):
            nc.vector.tensor_tensor(
                dest[0:DH, dsl], ps[0:DH, :],
                bias6[:, bcol:bcol + 1].to_broadcast((DH, 512)), ADD,
            )

        # ---- phase 1a: qT/kT/vT over nodes ----
        for i, dest in enumerate((QT, KT, vT)):
            for t2 in range(N // 512):
                ps = avp.tile([128, 512], F32, tag="av", name=f"ps1a_{i}_{t2}")
                nc.tensor.matmul(ps[0:DH, :], wqkv[:, i * DH:(i + 1) * DH],
                                 nodesT[:, t2 * 512:(t2 + 1) * 512],
                                 start=True, stop=True)
                proj_copy(ps, dest, slice(t2 * 512, (t2 + 1) * 512), i)

        # ---- phase 1b: e_qT/e_kT/e_vT over edges ----
        for i, dest in enumerate((QT, KT, vT)):
            for e2 in range(E // 512):
                sl = slice(e2 * 512, (e2 + 1) * 512)
                ps = avp.tile([128, 512], F32, tag="av", name=f"ps1b_{i}_{e2}")
                nc.tensor.matmul(ps[0:DH, :], eAB[:, 2 * i * DH:(2 * i + 1) * DH],
                                 srcT[:, sl], start=True, stop=False)
                nc.tensor.matmul(ps[0:DH, :], eAB[:, (2 * i + 1) * DH:(2 * i + 2) * DH],
                                 dstT[:, sl], start=False, stop=False)
                nc.tensor.matmul(ps[0:DH, :], eC[:, i * DH:(i + 1) * DH],
                                 efT[:, sl], start=False, stop=True)
                proj_copy(ps, dest, slice(N + e2 * 512, N + (e2 + 1) * 512), 3 + i)

        # ---- V natural via PE transpose of vT; ones column via memset ----
        nc.vector.memset(
            Vn[:].rearrange("p (c d) -> p c d", d=DH1)[:, :, DH:DH1], 1.0
        )
        for c in range(NJ):
            tp = smp.tile([128, DH], BF16, tag="sm", name=f"tp_{c}")
            nc.tensor.transpose(tp[:], vT[:, c * 128:(c + 1) * 128], ident[0:DH, 0:DH])
            nc.vector.tensor_copy(Vn[:, c * DH1:c * DH1 + DH], tp[:])

        # ---- duplicate QT/KT lo -> hi (for row-packed QK matmuls) ----
        if _no_sb2sb:
            nc.vector.memset(QT[DH:128, :], 0.0)
            nc.vector.memset(KT[DH:128, :], 0.0)
        else:
            nc.sync.dma_start(QT[DH:128, :], QT[0:DH, :])
            nc.sync.dma_start(KT[DH:128, :], KT[0:DH, :])

        # ---- phase 2: attention ----
        for m in range(NM):
            mq = slice(m * 512, (m + 1) * 512)
            pts = []
            for g in range(NJ // 2):
                cA, cB = 2 * g, 2 * g + 1
                st = stp.tile([128, 1024], F32, tag="st", name=f"st_{m}_{g}")
                nc.tensor.matmul(st[:, 0:512], KT[0:DH, cA * 128:(cA + 1) * 128],
                                 QT[0:DH, mq], start=True, stop=True)
                nc.tensor.matmul(st[:, 512:1024], KT[DH:128, cB * 128:(cB + 1) * 128],
                                 QT[DH:128, mq], start=True, stop=True)
                pt = ptp.tile([128, 1024], BF16, tag="pt", name=f"pt_{m}_{g}")
                nc.scalar.activation(pt[:], st[:], AF.Exp)
                pts.append(pt)
            av = avp.tile([128, 512], F32, tag="av", name=f"av_{m}")
            for c in range(NJ):
                pt = pts[c // 2]
                half = (c % 2) * 512
                nc.tensor.matmul(av[0:DH1, :], Vn[:, c * DH1:(c + 1) * DH1],
                                 pt[:, half:half + 512],
                                 start=(c == 0), stop=(c == NJ - 1))
            nc.vector.tensor_copy(OT[:, mq], av[0:DH, :])
            nc.vector.tensor_copy(rrow[0:1, mq], av[DH:DH1, :])
            nc.vector.reciprocal(rcp[0:1, mq], rrow[0:1, mq])

            # ---- normalize + output projection ----
            for t in range(4):
                tr = m * 4 + t
                ts_ = slice(tr * 128, (tr + 1) * 128)
                sm = smp.tile([128, 192], F32, tag="sm", name=f"sm_{tr}")
                w = wout[:, 0:DIM] if tr < N // 128 else wout[:, DIM:2 * DIM]
                nc.tensor.matmul(sm[:, 0:DIM], OT[:, ts_], w, start=True, stop=True)
                nc.tensor.matmul(sm[:, DIM:DIM + 1], rcp[0:1, ts_], ones[0:1, 0:1],
                                 start=True, stop=True)
                rtsb = work.tile([128, 1], F32, tag="rt", name=f"rt_{tr}")
                nc.vector.tensor_copy(rtsb[:], sm[:, DIM:DIM + 1])
                osb = work.tile([128, DIM], F32, tag="osb", name=f"osb_{tr}")
                nc.vector.tensor_scalar_mul(osb[:], sm[:, 0:DIM], rtsb[:])
                nc.sync.dma_start(d_out[ts_, :], osb[:])

    nc.compile()
    return nc


def _get_program():
    global _PROGRAM
    if _PROGRAM is None:
        _PROGRAM = _build_program()
    return _PROGRAM


def _prepare_in_maps(nodes, edge_features, edge_index, params):
    nodes = np.asarray(nodes, dtype=np.float32).reshape(N, DIM)
    ef = np.asarray(edge_features, dtype=np.float32).reshape(E, EFD)
    ei = np.asarray(edge_index).reshape(2, E).astype(np.int64)
    p = {k: np.asarray(v, dtype=np.float32) for k, v in params.items()}

    q_scale = np.float32(DH ** -0.5)
    c = np.float32((1.0 - ALPHA) * np.float32(np.sqrt(2.0).astype(np.float32)))
    s = np.float32(np.sqrt(DIM) / np.sqrt(2 * DIM))

    WQ = p["q_w"] * q_scale
    BQ = p["q_b"] * q_scale
    WK, BK = p["k_w"], p["k_b"]
    WV, BV = p["v_w"], p["v_b"]

    def fold(we_w, we_b, base_w, base_b, cross_w, cross_b, scale):
        A = scale * (c * we_w[:DIM] + ALPHA * base_w)
        B = scale * (c * we_w[DIM:] + ALPHA * base_w)
        C = scale * cross_w
        b = scale * (c * we_b + 2.0 * ALPHA * base_b + cross_b)
        return A, B, C, b

    AQ, BQe, CQ, bQe = fold(p["eq_w"], p["eq_b"], p["k_w"], p["k_b"],
                            p["kw_w"], p["kw_b"], s * q_scale)
    AK, BKe, CK, bKe = fold(p["ek_w"], p["ek_b"], p["q_w"], p["q_b"],
                            p["vw_w"], p["vw_b"], s)
    AV_, BVe, CV, bVe = fold(p["ev_w"], p["ev_b"], p["v_w"], p["v_b"],
                             p["qw_w"], p["qw_b"], s)

    nodes_bf = nodes.astype(_BF16)
    efT_bf = np.zeros((DIM, E), dtype=_BF16)
    efT_bf[:EFD] = ef.T.astype(_BF16)

    idx = np.empty((2, 128, E // 128), dtype=np.int32)
    for x in range(2):
        idx[x] = ei[x].reshape(E // 128, 128).T.astype(np.int32)

    def padC(Cm):  # [EFD, INNER] -> [DIM, INNER] zero-padded contraction
        out = np.zeros((DIM, Cm.shape[1]), np.float32)
        out[:EFD] = Cm
        return out

    CQp, CKp, CVp = padC(CQ), padC(CK), padC(CV)

    in_maps = []
    for h in range(H):
        sl = slice(h * DH, (h + 1) * DH)
        wqkv = np.concatenate([WQ[:, sl], WK[:, sl], WV[:, sl]], axis=1)
        eAB = np.concatenate([AQ[:, sl], BQe[:, sl], AK[:, sl], BKe[:, sl],
                              AV_[:, sl], BVe[:, sl]], axis=1)
        eCm = np.concatenate([CQp[:, sl], CKp[:, sl], CVp[:, sl]], axis=1)
        wo = np.concatenate([p["o_w"][sl, :], p["eo_w"][sl, :]], axis=1)
        bias6 = np.stack([BQ[sl], BK[sl], BV[sl], bQe[sl], bKe[sl], bVe[sl]],
                         axis=1)
        in_maps.append({
            "nodes_bf": np.ascontiguousarray(nodes_bf),
            "efT_bf": np.ascontiguousarray(efT_bf),
            "idx_i32": np.ascontiguousarray(idx),
            "wqkv": np.ascontiguousarray(wqkv.astype(_BF16)),
            "eAB": np.ascontiguousarray(eAB.astype(_BF16)),
            "eC": np.ascontiguousarray(eCm.astype(_BF16)),
            "wout": np.ascontiguousarray(wo.astype(_BF16)),
            "bias6": np.ascontiguousarray(bias6.astype(np.float32)),
        })
    return in_maps, p


def run(nodes, edge_features, edge_index, params, trace=False, tmpdir=None):
    """Run the SPMD kernel; returns ((node_out, edge_out), BassKernelResults)."""
    from concourse.bass_utils import run_bass_kernel_spmd

    in_maps, p = _prepare_in_maps(nodes, edge_features, edge_index, params)
    nc = _get_program()
    res = run_bass_kernel_spmd(
        nc, in_maps, core_ids=list(range(N_CORES)), trace=trace, tmpdir=tmpdir
    )
    acc = np.zeros((T, DIM), dtype=np.float32)
    for r in res.results:
        acc += r["out_partial"]
    node_out = (acc[:N] + p["o_b"]).astype(np.float32)[None]
    edge_out = (acc[N:] + p["eo_b"]).astype(np.float32)[None]
    return (node_out, edge_out), res


def kernel(nodes, edge_features, edge_index, params):
    out, _ = run(nodes, edge_features, edge_index, params, trace=False)
    return out
